# revision 1
# baseline (speedup 1.0000x reference)
"""RankLoss Trainium2 kernel.

Math: the reference loss per row reduces to per-row statistics of the three
logit matrices (no full softmax / top-k / sort needed).  Everything is
expressed in the target-shifted exp domain u = exp(x - x[target]) (the
logits are standard-normal scale, so exp never overflows):
  for each classifier x in {sub, rel, obj}:
    u1 = max(u), u2 = second max(u)   (one DVE max8 pass over u)
    W  = sum(u) = Z/exp(x[target])    (ACT exp pass, bias=-x[target],
                                       accumulate)
    argmax == target  <=>  u1 == 1.0  (u[target] = exp(0) = 1 exactly)
    top-1 prob = u1/W, top-2 prob = u2/W, target prob = 1/W
  invP = 1/(Ws*Wr*Wo)
  gt   = invP
  top1 = u1s*u1r*u1o*invP
  second-smallest of the 8 top-2 products
       = invP * min(u1s*u2r*u2o, u2s*u1r*u2o, u2s*u2r*u1o)
    (the smallest is u2s*u2r*u2o; every other of the 8 products dominates
     one of those three corners.)
  pre  = cond ? second_smallest : top1
  loss = mean(relu(1 - gt + pre))

Per core (pure data parallel over the batch): 32 tiles x [128, C] per input.
Per tile: one ACT exp pass (bias = -x_t, with W accumulate) and one DVE max8
pass; bulk loads stream on the sync HWDGE queue.  The target logits x_t are
pre-gathered on the host in make_in_maps (index plumbing, 4 KB/input/core;
the HW indirect-DMA gather costs +60 us/rep because it fetches one element
per partition per DMA and its dynamic queue serializes against the stream)
and loaded with one contiguous DMA on the otherwise-idle gpsimd queue.
Final math on [128, 32] stat tiles, partition all-reduce, partial sum out.
Host sums the 8 per-core partials (the unshard step).

The steady-state rate is HBM-bound: the 8 cores share ~2.7 TB/s, so the
41 MB/core input stream sets a ~120 us/rep floor; ACT (~108 us) and DVE
(~96 us) passes fit underneath it.
"""

import numpy as np

B = 32768
N_CORES = 8
B_CORE = B // N_CORES  # 4096
P = 128
NT = B_CORE // P  # 32
C_ENT = 1000
C_REL = 500
INV_B = 1.0 / B

SPECS = [("sub", C_ENT), ("rel", C_REL), ("obj", C_ENT)]

# which engine's HWDGE queue carries each input's streaming loads
DMA_ENGINE = {"sub": "sync", "obj": "sync", "rel": "sync"}
# tiles of 128 rows per DMA chunk (contiguous per partition thanks to the
# row = p*NT + n layout); knobs for data/exp-scratch pool depths
CHUNK = 1
DATA_BUFS = 6
E_BUFS = 5
# e-tile dtype for the max8 path ("f32" or "bf16")
E_DTYPE = "f32"
# compute exp in place on the data tile (f32 only; frees the e pool so
# larger CHUNKs fit deep buffering)
E_INPLACE = False
# stream order: "mk" interleaves the three inputs per chunk; "km" streams
# each input's chunks back-to-back.  km's sequential 16 MB per-tensor DRAM
# reads measure ~13% faster than interleaved (106 vs 123 us/rep, bracketed
# A/B): DRAM page locality dominates.
STREAM_ORDER = "km"
# engine for the rel input's Z accumulation: "act" fuses it into the exp
# pass (costs ~190ns/tile of ACT accumulator reads); "pool" frees ACT by
# summing e_rel on the otherwise-idle gpsimd engine
Z_REL = "act"
# emit the gather block after this many stream chunks (0 = before the
# stream).  The gathers feed the ACT bias, so they must land well before
# the exp of the first tile; they run on the gpsimd queue to stay off the
# stream's sync queue.
GATHER_AT = 0
# timing-only ablations (break correctness): subset of
# {"gather","max8","exp","final","stream"}
ABLATE = set()
# dump per-row stats (W, u1, u2, xt) as extra outputs for debugging
DEBUG_DUMP = False

_cache = {}


def _build(reps: int = 1, loops: int = 1):
    import contextlib

    import concourse.bacc as bacc
    import concourse.mybir as mybir
    import concourse.tile as tile
    from concourse import bass_isa

    f32 = mybir.dt.float32
    i32 = mybir.dt.int32
    e_dt = mybir.dt.bfloat16 if E_DTYPE == "bf16" else f32
    Exp = mybir.ActivationFunctionType.Exp
    Alu = mybir.AluOpType

    nc = bacc.Bacc("TRN2", target_bir_lowering=False, debug=False,
                   enable_asserts=False)

    x_d, g_d = {}, {}
    for k, C in SPECS:
        x_d[k] = nc.dram_tensor(f"x_{k}", [B_CORE, C], f32, kind="ExternalInput")
        g_d[k] = nc.dram_tensor(f"g_{k}", [B_CORE], f32, kind="ExternalInput")
    out_d = nc.dram_tensor("partial", [1, 1], f32, kind="ExternalOutput")
    dbg_d = {}
    if DEBUG_DUMP:
        for k, _ in SPECS:
            for nmv in ("W", "u1", "u2", "xt"):
                dbg_d[f"{nmv}_{k}"] = nc.dram_tensor(
                    f"dbg_{nmv}_{k}", [P, NT], f32, kind="ExternalOutput")

    dma_engine = dict(DMA_ENGINE)

    with tile.TileContext(nc) as tc:
        with (
            tc.tile_pool(name="stats", bufs=2 if reps > 1 else 1) as st,
            tc.tile_pool(name="data", bufs=DATA_BUFS) as dp,
            tc.tile_pool(name="escratch", bufs=E_BUFS) as ep,
            tc.tile_pool(name="fin", bufs=2 if reps > 1 else 1) as fp,
            (tc.For_i(0, loops) if loops > 1 else contextlib.nullcontext()),
        ):
          for _rep in range(reps):
            top8 = {k: st.tile([P, NT, 8], e_dt, tag=f"top8_{k}",
                               name=f"top8_{k}")
                    for k, _ in SPECS}
            zsum = {k: st.tile([P, NT], f32, tag=f"z_{k}", name=f"z_{k}")
                    for k, _ in SPECS}
            xtn = {k: st.tile([P, NT], f32, tag=f"xtn_{k}", name=f"xtn_{k}")
                   for k, _ in SPECS}

            if ABLATE:
                for k, _ in SPECS:
                    nc.vector.memset(top8[k][:, :, :], 0.5)
                    nc.vector.memset(zsum[k][:, :], 1.0)
                    nc.vector.memset(xtn[k][:, :], 0.5)

            # Target logits x[row, target[row]] are pre-gathered on the host
            # (pure index plumbing in make_in_maps; 4 KB per input per core)
            # and loaded with one contiguous DMA on the idle gpsimd queue,
            # then negated: the ACT pass uses bias = -x_t.  The device-side
            # alternative (indirect DMA) costs +60 us/rep: the HW primitive
            # gathers one element per partition per DMA (96 DMAs/rep) and
            # qPoolDynamic doesn't overlap the stream.  Row layout:
            # row = p*NT + n (partition p, stat column n).
            def emit_gather():
              for k, C in SPECS if "gather" not in ABLATE else []:
                xt = st.tile([P, NT], f32, tag=f"xt_{k}", name=f"xt_{k}")
                nc.gpsimd.dma_start(
                    out=xt[:, :],
                    in_=g_d[k].ap().rearrange("(p n) -> p n", p=P),
                )
                nc.gpsimd.tensor_scalar_mul(xtn[k][:, :], xt[:, :], -1.0)

            # Main streaming loop: CHUNK tiles per DMA; per tile one ACT
            # exp/accum (bias = -x_t) + one DVE max8.
            CH = CHUNK
            xv = {k: x_d[k].ap().rearrange("(p m u) c -> m p (u c)",
                                           p=P, m=NT // CH, u=CH)
                  for k, _ in SPECS}
            if GATHER_AT == 0:
                emit_gather()
            n_chunks = NT // CH if "stream" not in ABLATE else 0
            if STREAM_ORDER == "km":
                sched = [(m, k, C) for k, C in SPECS for m in range(n_chunks)]
            else:
                sched = [(m, k, C) for m in range(n_chunks) for k, C in SPECS]
            for m, k, C in sched:
                if True:
                    xtile = dp.tile([P, CH * C], f32, tag=f"x_{k}",
                                    name=f"xt_{k}_{m}")
                    getattr(nc, dma_engine[k]).dma_start(
                        out=xtile[:, :], in_=xv[k][m])
                    e = xtile if E_INPLACE else ep.tile(
                        [P, CH * C], e_dt, tag=f"e_{k}", name=f"e_{k}_{m}")
                    for u in range(CH):
                        n = m * CH + u
                        cs = slice(u * C, (u + 1) * C)
                        zpool = Z_REL == "pool" and k == "rel"
                        if "exp" not in ABLATE:
                            nc.scalar.activation(
                                out=e[:, cs], in_=xtile[:, cs], func=Exp,
                                bias=(0.0 if "gather" in ABLATE
                                      else xtn[k][:, n:n + 1]),
                                accum_out=(None if zpool
                                           else zsum[k][:, n:n + 1]),
                            )
                            if zpool:
                                zj = ep.tile([P, C], f32, tag="zjunk",
                                             name=f"zjunk_{n}")
                                # accum (TensorScalarPtrReduce) requires an
                                # explicit op1
                                nc.gpsimd.tensor_scalar(
                                    zj[:, :], e[:, cs], 0.0, None,
                                    op0=Alu.add, op1=Alu.add,
                                    accum_out=zsum[k][:, n:n + 1])
                        if "max8" not in ABLATE:
                            nc.vector.max(
                                out=top8[k][:, n, :],
                                in_=(e if "exp" not in ABLATE
                                     else xtile)[:, cs])
                        elif "exp" in ABLATE:
                            # tiny consumer so the load isn't dead
                            nc.vector.tensor_scalar_mul(
                                zsum[k][:, 0:1],
                                xtile[:, u * C:u * C + 1], 1.0)
                if (GATHER_AT not in (None, 0) and m + 1 == GATHER_AT
                        and k == SPECS[-1][0]):
                    emit_gather()

            # Final math on [P, NT] stat tiles.
            if "final" in ABLATE:
                ptot0 = fp.tile([P, 1], f32, tag="ptot", name="ptot")
                nc.vector.memset(ptot0[:, :], 0.0)
                nc.sync.dma_start(out=out_d[:, :], in_=ptot0[0:1, 0:1])
                continue

            def ft(tag):
                return fp.tile([P, NT], f32, tag=tag, name=tag)

            if DEBUG_DUMP:
                for k, _ in SPECS:
                    nc.sync.dma_start(out=dbg_d[f"W_{k}"].ap(),
                                      in_=zsum[k][:, :])
                    for nmv, idx in (("u1", 0), ("u2", 1)):
                        cp = ft(f"dbg_{nmv}_{k}")
                        nc.vector.tensor_scalar_mul(
                            cp[:, :], top8[k][:, :, idx], 1.0)
                        nc.sync.dma_start(out=dbg_d[f"{nmv}_{k}"].ap(),
                                          in_=cp[:, :])
                    cpx = ft(f"dbg_xt_{k}")
                    nc.vector.tensor_scalar_mul(cpx[:, :], xtn[k][:, :], -1.0)
                    nc.sync.dma_start(out=dbg_d[f"xt_{k}"].ap(),
                                      in_=cpx[:, :])

            u1 = {k: top8[k][:, :, 0] for k, _ in SPECS}
            u2 = {k: top8[k][:, :, 1] for k, _ in SPECS}

            # cond[k]: argmax == target  <=>  u1 == 1.0
            cnd = {}
            for k, _ in SPECS:
                cnd[k] = ft(f"cnd_{k}")
                nc.vector.tensor_scalar(
                    out=cnd[k][:, :], in0=u1[k][:, :], scalar1=1.0,
                    scalar2=None, op0=Alu.is_equal)

            # invP = 1/(Ws*Wr*Wo);  gt = invP
            zp = ft("zp")
            nc.vector.tensor_mul(zp[:, :], zsum["sub"][:, :], zsum["rel"][:, :])
            nc.vector.tensor_mul(zp[:, :], zp[:, :], zsum["obj"][:, :])
            invp = ft("invp")
            nc.vector.reciprocal(invp[:, :], zp[:, :])

            # top-1 product and the three "one top-1, two top-2" corners
            t1 = ft("t1")
            nc.vector.tensor_mul(t1[:, :], u1["sub"][:, :], u1["rel"][:, :])
            nc.vector.tensor_mul(t1[:, :], t1[:, :], u1["obj"][:, :])

            mn = ft("mn")
            tmp = ft("tmp")
            # corner_sub = u1s*u2r*u2o
            nc.vector.tensor_mul(mn[:, :], u2["rel"][:, :], u2["obj"][:, :])
            nc.vector.tensor_mul(mn[:, :], mn[:, :], u1["sub"][:, :])
            # corner_rel = u2s*u1r*u2o
            nc.vector.tensor_mul(tmp[:, :], u2["sub"][:, :], u2["obj"][:, :])
            nc.vector.tensor_mul(tmp[:, :], tmp[:, :], u1["rel"][:, :])
            nc.vector.tensor_tensor(out=mn[:, :], in0=mn[:, :], in1=tmp[:, :],
                                    op=Alu.min)
            # corner_obj = u2s*u2r*u1o
            nc.vector.tensor_mul(tmp[:, :], u2["sub"][:, :], u2["rel"][:, :])
            nc.vector.tensor_mul(tmp[:, :], tmp[:, :], u1["obj"][:, :])
            nc.vector.tensor_tensor(out=mn[:, :], in0=mn[:, :], in1=tmp[:, :],
                                    op=Alu.min)

            cond = ft("cond")
            nc.vector.tensor_mul(cond[:, :], cnd["sub"][:, :], cnd["rel"][:, :])
            nc.vector.tensor_mul(cond[:, :], cond[:, :], cnd["obj"][:, :])

            # pre_num = t1 + cond*(mn - t1);
            # out = relu(1 + invp*(pre_num - 1))
            nc.vector.tensor_sub(mn[:, :], mn[:, :], t1[:, :])
            nc.vector.tensor_mul(mn[:, :], mn[:, :], cond[:, :])
            nc.vector.tensor_add(mn[:, :], mn[:, :], t1[:, :])
            nc.vector.tensor_scalar_add(mn[:, :], mn[:, :], -1.0)
            nc.vector.tensor_mul(mn[:, :], mn[:, :], invp[:, :])
            nc.vector.tensor_scalar_add(mn[:, :], mn[:, :], 1.0)

            relu = ft("relu")
            rowsum = fp.tile([P, 1], f32, tag="rowsum", name="rowsum")
            nc.vector.tensor_scalar(relu[:, :], mn[:, :], 0.0, None,
                                    op0=Alu.max, op1=Alu.add,
                                    accum_out=rowsum[:, :])
            nc.vector.tensor_scalar_mul(rowsum[:, :], rowsum[:, :], INV_B)
            ptot = fp.tile([P, 1], f32, tag="ptot", name="ptot")
            nc.gpsimd.partition_all_reduce(
                ptot[:, :], rowsum[:, :], channels=P,
                reduce_op=bass_isa.ReduceOp.add)
            nc.sync.dma_start(out=out_d[:, :], in_=ptot[0:1, 0:1])

    nc.compile()
    return nc


def _get_nc(reps: int = 1, loops: int = 1):
    key = ("nc", reps, loops)
    if key not in _cache:
        _cache[key] = _build(reps, loops)
    return _cache[key]


def make_in_maps(sub_input, relation_input, obj_input,
                 sub_target, relation_target, obj_target):
    arrs = {
        "x_sub": np.ascontiguousarray(np.asarray(sub_input, dtype=np.float32)),
        "x_rel": np.ascontiguousarray(np.asarray(relation_input, dtype=np.float32)),
        "x_obj": np.ascontiguousarray(np.asarray(obj_input, dtype=np.float32)),
    }
    tgts = {
        "sub": np.asarray(sub_target).astype(np.int64),
        "rel": np.asarray(relation_target).astype(np.int64),
        "obj": np.asarray(obj_target).astype(np.int64),
    }
    rows = np.arange(B_CORE)
    in_maps = []
    for c in range(N_CORES):
        lo, hi = c * B_CORE, (c + 1) * B_CORE
        m = {k: np.ascontiguousarray(v[lo:hi]) for k, v in arrs.items()}
        for (k, C) in SPECS:
            # pre-gather the target logit per row (index plumbing only; all
            # loss arithmetic happens on device)
            xk = m[f"x_{k}"]
            m[f"g_{k}"] = np.ascontiguousarray(xk[rows, tgts[k][lo:hi]])
        in_maps.append(m)
    return in_maps


def run_spmd(in_maps, **kwargs):
    from concourse.bass_utils import run_bass_kernel_spmd
    nc = _get_nc()
    return run_bass_kernel_spmd(nc, in_maps, core_ids=list(range(N_CORES)),
                                **kwargs)


def kernel(sub_input, relation_input, obj_input,
           sub_target, relation_target, obj_target):
    in_maps = make_in_maps(sub_input, relation_input, obj_input,
                           sub_target, relation_target, obj_target)
    res = run_spmd(in_maps)
    total = np.float64(0.0)
    for r in res.results:
        total += np.float64(r["partial"].reshape(-1)[0])
    return np.float32(total)

